# revision 5
# baseline (speedup 1.0000x reference)
"""CIM signed-magnitude linear kernel v3 — fp16 transport, fp16 stores.

Math identity (exact): y = (x_q @ w_q.T) * scale_x * scale_w.T + bias with
x_q = round(x / (max|x|/127 + eps)) per token, w_q likewise per out-channel.
(The reference's bit-serial/ADC path is numerically identical: per-chunk
partial sums are <= 64 so the [0,1023] ADC clamp never binds and rounding of
integers is exact.)

v3 vs v2:
  * fp16 HBM transport for x / w / bias (host cast) — input DMA halves to
    ~2MB/core.  fp16 keeps 11 significand bits; cast error ~2^-11 per
    element, far inside the 2e-2 rel-err budget.
  * fp16 output stores ([OC, TC] per core, host-transposed + f32 cast) —
    store traffic halves.
  * One-pass quantize unchanged: q_fp16 = (x * xinv) + 1536.0; the single
    f32->fp16 rounding lands on the integer grid (fp16 ulp is exactly 1.0 on
    [1024, 2048)); the -1536 de-bias rides the mandatory PSUM->SBUF evict.
  * Evicts operate on full PSUM banks ([128, 1024] fp16 = 8 transposes) to
    amortize per-op overheads; split ACT/DVE.
  * Engine split: DVE = x-reduces + recips + out-TT + evicts; Pool =
    w-reduces + quant halves + out-TS; ACT = quant halves + evicts.
  * mm groups split by token halves so PE starts real MMs as soon as the
    first w tile lands.

Sharding: 8 cores = 4 token-shards x 2 out-feature shards, no collectives.
"""

import os

os.environ.setdefault("JAX_PLATFORMS", "cpu")

import numpy as np

B, S, IN_F, OUT_F = 2, 1024, 1024, 1024
T = B * S
M_SHARDS, N_SHARDS = 4, 2
TC = T // M_SHARDS             # 512 tokens per core
OC = OUT_F // N_SHARDS         # 512 out-features per core
NT = TC // 128                 # 4 token tiles
NO = OC // 128                 # 4 out-feature tiles
KB = IN_F // 128               # 8 contraction blocks
WROW = IN_F + 1                # weight row with bias appended

EPS = 1e-8
INV127 = 1.0 / 127.0
INV16129 = 1.0 / 16129.0
# fp16 has 10 mantissa bits: on [1024,2048) the ulp is exactly 1.0, so the
# single f32->fp16 output rounding of (q + 1536) lands on the integer grid.
MAGIC = 1536.0
N_WARMUP = 20

_CACHE = {}


def _build_nc():
    import concourse.bass as bass
    import concourse.mybir as mybir
    import concourse.tile as tile
    from concourse.masks import make_identity

    F32 = mybir.dt.float32
    F16 = mybir.dt.float16
    ALU = mybir.AluOpType
    ACTF = mybir.ActivationFunctionType
    AX = mybir.AxisListType

    nc = bass.Bass("TRN2", target_bir_lowering=False, debug=False)

    x_d = nc.dram_tensor("x", [TC, IN_F], F16, kind="ExternalInput").ap()
    wb_d = nc.dram_tensor("wb", [OC, WROW], F16, kind="ExternalInput").ap()
    out_d = nc.dram_tensor("out", [OC, TC], F16, kind="ExternalOutput").ap()

    x3 = x_d.rearrange("(q p) i -> p q i", p=128)     # [128, NT, IN_F]
    w3 = wb_d.rearrange("(r p) i -> p r i", p=128)    # [128, NO, WROW]

    with tile.TileContext(nc) as tc:
        with (
            tc.tile_pool(name="raw", bufs=1) as raw,
            tc.tile_pool(name="qb", bufs=1) as qb,
            tc.tile_pool(name="small", bufs=1) as small,
            tc.tile_pool(name="ob", bufs=2) as obp,
            tc.tile_pool(name="mmps", bufs=4, space="PSUM") as mmps,
            tc.tile_pool(name="stps", bufs=2, space="PSUM") as stps,
            tc.tile_pool(name="auxps", bufs=1, space="PSUM") as auxps,
        ):
            x_sb = raw.tile([128, NT, IN_F], F16, tag="x_sb")
            w_sb = raw.tile([128, NO, WROW], F16, tag="w_sb")
            xq = qb.tile([128, NT, IN_F], F16, tag="xq")      # values +1536
            wq = qb.tile([128, NO, IN_F], F16, tag="wq")      # values +1536
            xqT = qb.tile([128, KB, TC], F16, tag="xqT")      # true ints
            wqT = qb.tile([128, KB, OC], F16, tag="wqT")      # true ints
            bcx = qb.tile([128, TC], F32, tag="bcx")          # sx/16129 bcast
            row_sb = qb.tile([1, TC], F32, tag="row_sb")
            ones1 = qb.tile([1, 128], F32, tag="ones1")
            ident_f16 = qb.tile([128, 128], F16, tag="ident_f16")
            ident_f = qb.tile([128, 128], F32, tag="ident_f")

            xmax = small.tile([128, NT], F32, tag="xmax")
            xden = small.tile([128, NT], F32, tag="xden")
            xinv = small.tile([128, NT], F32, tag="xinv")
            wmax = small.tile([128, NO], F32, tag="wmax")
            wmaxh = small.tile([128, 2], F32, tag="wmaxh")    # w3 half-maxes
            wden = small.tile([128, NO], F32, tag="wden")
            winv = small.tile([128, NO], F32, tag="winv")
            bias2 = small.tile([128, NO], F32, tag="bias2")

            ps = [mmps.tile([128, TC], F32, tag="ps", name=f"ps{m}")
                  for m in range(NO)]
            st = [stps.tile([128, 1024], F16, tag="st", name=f"st{i}")
                  for i in range(2)]
            rowps = auxps.tile([128, TC], F32, tag="rowps")   # xmax row stage
            bcps = auxps.tile([128, TC], F32, tag="bcps")     # bcx broadcast

            # ---- constants ----
            nc.gpsimd.memset(ones1, 1.0)
            make_identity(nc, ident_f16)
            make_identity(nc, ident_f)

            # ---- loads: x tiles first, w tiles after, w3 split ----
            for q in range(NT):
                nc.sync.dma_start(out=x_sb[:, q:q + 1], in_=x3[:, q:q + 1])
            for r in range(NO - 1):
                nc.sync.dma_start(out=w_sb[:, r:r + 1], in_=w3[:, r:r + 1])
            nc.sync.dma_start(out=w_sb[:, 3, 0:512], in_=w3[:, 3, 0:512])
            nc.sync.dma_start(out=w_sb[:, 3, 512:WROW], in_=w3[:, 3, 512:WROW])

            # ---- PE warmup: ramp the clock while loads stream ----
            for i in range(N_WARMUP):
                nc.tensor.transpose(st[0][:, 0:128], ident_f16, ident_f16)

            def x_red(q):
                nc.vector.tensor_reduce(
                    out=xmax[:, q:q + 1], in_=x_sb[:, q, :], axis=AX.X,
                    op=ALU.max, apply_absolute_value=True)
                nc.vector.tensor_scalar(
                    out=xden[:, q:q + 1], in0=xmax[:, q:q + 1],
                    scalar1=INV127, scalar2=EPS, op0=ALU.mult, op1=ALU.add)
                nc.vector.reciprocal(out=xinv[:, q:q + 1], in_=xden[:, q:q + 1])

            def x_quant_act(q, c0, c1):
                nc.scalar.activation(
                    out=xq[:, q, c0:c1], in_=x_sb[:, q, c0:c1], func=ACTF.Copy,
                    scale=xinv[:, q:q + 1], bias=MAGIC)

            def x_quant_pool(q, c0, c1):
                nc.gpsimd.tensor_scalar(
                    out=xq[:, q, c0:c1], in0=x_sb[:, q, c0:c1],
                    scalar1=xinv[:, q:q + 1], scalar2=MAGIC,
                    op0=ALU.mult, op1=ALU.add)

            def x_T(q):
                # all 8 contraction blocks of token-tile q into one st bank
                bank = st[q % 2]
                for k in range(KB):
                    nc.tensor.transpose(
                        bank[:, k * 128:(k + 1) * 128],
                        xq[:, q, k * 128:(k + 1) * 128], ident_f16)
                return bank

            def x_ev(q, bank, eng):
                # full-bank evict: st bank -> xqT[:, :, q-block], de-bias
                out_ap = xqT[:, 0:KB, q * 128:(q + 1) * 128]
                in_ap = bank.rearrange("p (k c) -> p k c", c=128)
                if eng == "act":
                    nc.scalar.activation(out=out_ap, in_=in_ap,
                                         func=ACTF.Copy, scale=1.0, bias=-MAGIC)
                else:
                    nc.vector.tensor_scalar(out=out_ap, in0=in_ap,
                                            scalar1=-MAGIC, scalar2=None,
                                            op0=ALU.add)

            def w_red(r):
                nc.vector.tensor_reduce(
                    out=wmax[:, r:r + 1], in_=w_sb[:, r, 0:IN_F], axis=AX.X,
                    op=ALU.max, apply_absolute_value=True)

            def w_den(r):
                nc.vector.tensor_scalar(
                    out=wden[:, r:r + 1], in0=wmax[:, r:r + 1],
                    scalar1=INV127, scalar2=EPS, op0=ALU.mult, op1=ALU.add)
                nc.vector.reciprocal(out=winv[:, r:r + 1], in_=wden[:, r:r + 1])

            def w_quant(r, c0, c1, eng):
                if eng == "act":
                    nc.scalar.activation(
                        out=wq[:, r, c0:c1], in_=w_sb[:, r, c0:c1],
                        func=ACTF.Copy, scale=winv[:, r:r + 1], bias=MAGIC)
                else:
                    nc.gpsimd.tensor_scalar(
                        out=wq[:, r, c0:c1], in0=w_sb[:, r, c0:c1],
                        scalar1=winv[:, r:r + 1], scalar2=MAGIC,
                        op0=ALU.mult, op1=ALU.add)

            def w_T(r):
                bank = st[r % 2]
                for k in range(KB):
                    nc.tensor.transpose(
                        bank[:, k * 128:(k + 1) * 128],
                        wq[:, r, k * 128:(k + 1) * 128], ident_f16)
                return bank

            def w_ev(r, bank, eng):
                out_ap = wqT[:, 0:KB, r * 128:(r + 1) * 128]
                in_ap = bank.rearrange("p (k c) -> p k c", c=128)
                if eng == "act":
                    nc.scalar.activation(out=out_ap, in_=in_ap,
                                         func=ACTF.Copy, scale=1.0, bias=-MAGIC)
                else:
                    nc.vector.tensor_scalar(out=out_ap, in0=in_ap,
                                            scalar1=-MAGIC, scalar2=None,
                                            op0=ALU.add)

            def mm_group(m):
                for k in range(KB):
                    nc.tensor.matmul(
                        ps[m], lhsT=wqT[:, k, m * 128:(m + 1) * 128],
                        rhs=xqT[:, k, :], start=(k == 0), stop=(k == KB - 1))

            def out_evict(m):
                otmp = obp.tile([128, TC], F16, tag="otmp", name=f"otmp{m}")
                nc.vector.tensor_tensor(out=otmp, in0=ps[m], in1=bcx,
                                        op=ALU.mult)
                osb = obp.tile([128, TC], F16, tag="osb", name=f"osb{m}")
                nc.gpsimd.tensor_scalar(
                    out=osb, in0=otmp, scalar1=wmax[:, m:m + 1],
                    scalar2=bias2[:, m:m + 1], op0=ALU.mult, op1=ALU.add)
                nc.sync.dma_start(out=out_d[m * 128:(m + 1) * 128, :], in_=osb)

            # ================= pipelined emission (approx time order) ======
            # --- x chains ---
            x_red(0)
            x_quant_act(0, 0, 512)
            x_quant_pool(0, 512, IN_F)
            x_T(0)
            x_red(1)
            x_quant_act(1, 0, 512)
            x_quant_pool(1, 512, IN_F)
            x_ev(0, st[0], "act")
            x_T(1)
            x_red(2)
            x_quant_act(2, 0, 512)
            x_quant_pool(2, 512, IN_F)
            x_ev(1, st[1], "dve")
            x_T(2)
            x_red(3)
            x_quant_act(3, 0, 512)
            x_quant_pool(3, 512, IN_F)
            x_ev(2, st[0], "act")
            x_T(3)
            # token-scale row: 4 single-column PE transposes into rowps
            for q in range(NT):
                nc.tensor.transpose(
                    rowps[0:1, q * 128:(q + 1) * 128], xmax[:, q:q + 1],
                    ident_f)
            nc.scalar.activation(
                out=row_sb, in_=rowps[0:1, :], func=ACTF.Copy,
                scale=INV16129, bias=0.0)
            nc.tensor.matmul(bcps, lhsT=ones1, rhs=row_sb,
                             start=True, stop=True)
            x_ev(3, st[1], "dve")
            nc.vector.tensor_copy(out=bcx, in_=bcps)

            # --- w chains ---
            w_red(0)
            w_den(0)
            w_quant(0, 0, 512, "act")
            w_quant(0, 512, IN_F, "pool")
            nc.gpsimd.tensor_copy(out=bias2[:, 0:1], in_=w_sb[:, 0, IN_F:WROW])
            w_T(0)
            w_red(1)
            w_den(1)
            w_ev(0, st[0], "act")
            w_quant(1, 0, 512, "act")
            w_quant(1, 512, IN_F, "pool")
            nc.gpsimd.tensor_copy(out=bias2[:, 1:2], in_=w_sb[:, 1, IN_F:WROW])
            w_T(1)
            mm_group(0)
            w_red(2)
            w_den(2)
            w_ev(1, st[1], "dve")
            w_quant(2, 0, 512, "act")
            w_quant(2, 512, IN_F, "pool")
            nc.gpsimd.tensor_copy(out=bias2[:, 2:3], in_=w_sb[:, 2, IN_F:WROW])
            w_T(2)
            mm_group(1)
            out_evict(0)
            # w3: reduce halves on DVE, den on DVE, quant ACT || Pool
            nc.vector.tensor_reduce(
                out=wmaxh[:, 0:1], in_=w_sb[:, 3, 0:512], axis=AX.X,
                op=ALU.max, apply_absolute_value=True)
            nc.vector.tensor_reduce(
                out=wmaxh[:, 1:2], in_=w_sb[:, 3, 512:IN_F], axis=AX.X,
                op=ALU.max, apply_absolute_value=True)
            nc.vector.tensor_tensor(
                out=wmax[:, 3:4], in0=wmaxh[:, 0:1], in1=wmaxh[:, 1:2],
                op=ALU.max)
            w_den(3)
            w_ev(2, st[0], "act")
            w_quant(3, 0, 512, "act")
            w_quant(3, 512, IN_F, "pool")
            nc.gpsimd.tensor_copy(out=bias2[:, 3:4], in_=w_sb[:, 3, IN_F:WROW])
            w_T(3)
            mm_group(2)
            out_evict(1)
            w_ev(3, st[1], "dve")
            mm_group(3)
            out_evict(2)
            out_evict(3)

    _split_multiwaits(nc)
    return nc


def _split_multiwaits(nc):
    """Hoist all but one wait of any multi-wait instruction into standalone
    EventSemaphore instructions (the ISA carries one wait per instruction)."""
    import concourse.mybir as mybir

    fn = nc.m.functions[0]
    ctr = [0]
    for blk in fn.blocks:
        insts = list(blk.instructions)
        changed = False
        out = []
        for inst in insts:
            si = inst.sync_info
            waits = list(si.on_wait or []) if si is not None else []
            if len(waits) > 1:
                for w in waits[:-1]:
                    ctr[0] += 1
                    es = mybir.InstEventSemaphore(
                        name=f"I-eswait-{ctr[0]}", engine=inst.engine,
                        ins=[], outs=[],
                    )
                    es.sync_info = mybir.SyncInfo(on_wait=[w], on_update=[])
                    out.append(es)
                    nc.register_instruction(es)
                inst.sync_info = mybir.SyncInfo(
                    on_wait=[waits[-1]], on_update=list(si.on_update or []),
                )
                changed = True
            out.append(inst)
        if changed:
            blk.instructions = out


def get_nc():
    if "nc" not in _CACHE:
        _CACHE["nc"] = _build_nc()
    return _CACHE["nc"]


def make_in_maps(x, weight, bias):
    xf = np.asarray(x, dtype=np.float32).reshape(T, IN_F).astype(np.float16)
    w = np.asarray(weight, dtype=np.float32).astype(np.float16)
    b = np.asarray(bias, dtype=np.float32).astype(np.float16)
    wb = np.concatenate([w, b[:, None]], axis=1)
    in_maps = []
    for c in range(M_SHARDS * N_SHARDS):
        im, jn = divmod(c, N_SHARDS)
        in_maps.append({
            "x": np.ascontiguousarray(xf[im * TC:(im + 1) * TC]),
            "wb": np.ascontiguousarray(wb[jn * OC:(jn + 1) * OC]),
        })
    return in_maps


def assemble(results):
    y = np.empty((T, OUT_F), dtype=np.float32)
    for c in range(M_SHARDS * N_SHARDS):
        im, jn = divmod(c, N_SHARDS)
        y[im * TC:(im + 1) * TC, jn * OC:(jn + 1) * OC] = \
            results[c]["out"].T.astype(np.float32)
    return y.reshape(B, S, OUT_F)


def run(x, weight, bias, **spmd_kwargs):
    from concourse.bass_utils import run_bass_kernel_spmd

    nc = get_nc()
    in_maps = make_in_maps(x, weight, bias)
    res = run_bass_kernel_spmd(nc, in_maps, core_ids=list(range(8)), **spmd_kwargs)
    return assemble(res.results), res


def kernel(x, weight, bias):
    y, _ = run(x, weight, bias)
    return y


# revision 14
# speedup vs baseline: 1.0700x; 1.0700x over previous
"""CIM signed-magnitude linear kernel v3 — fp16 transport, fp16 stores.

Math identity (exact): y = (x_q @ w_q.T) * scale_x * scale_w.T + bias with
x_q = round(x / (max|x|/127 + eps)) per token, w_q likewise per out-channel.
(The reference's bit-serial/ADC path is numerically identical: per-chunk
partial sums are <= 64 so the [0,1023] ADC clamp never binds and rounding of
integers is exact.)

v3 vs v2:
  * fp16 HBM transport for x / w / bias (host cast) — input DMA halves to
    ~2MB/core.  fp16 keeps 11 significand bits; cast error ~2^-11 per
    element, far inside the 2e-2 rel-err budget.
  * fp16 output stores ([OC, TC] per core, host-transposed + f32 cast) —
    store traffic halves.
  * One-pass quantize unchanged: q_fp16 = (x * xinv) + 1536.0; the single
    f32->fp16 rounding lands on the integer grid (fp16 ulp is exactly 1.0 on
    [1024, 2048)); the -1536 de-bias rides the mandatory PSUM->SBUF evict.
  * Evicts operate on full PSUM banks ([128, 1024] fp16 = 8 transposes) to
    amortize per-op overheads; split ACT/DVE.
  * Engine split: DVE = x-reduces + recips + out-TT + evicts; Pool =
    w-reduces + quant halves + out-TS; ACT = quant halves + evicts.
  * mm groups split by token halves so PE starts real MMs as soon as the
    first w tile lands.

Sharding: 8 cores = 4 token-shards x 2 out-feature shards, no collectives.
"""

import os

os.environ.setdefault("JAX_PLATFORMS", "cpu")

import numpy as np

B, S, IN_F, OUT_F = 2, 1024, 1024, 1024
T = B * S
M_SHARDS, N_SHARDS = 4, 2
TC = T // M_SHARDS             # 512 tokens per core
OC = OUT_F // N_SHARDS         # 512 out-features per core
NT = TC // 128                 # 4 token tiles
NO = OC // 128                 # 4 out-feature tiles
KB = IN_F // 128               # 8 contraction blocks
WROW = IN_F + 1                # weight row with bias appended

EPS = 1e-8
INV127 = 1.0 / 127.0
INV16129 = 1.0 / 16129.0
# fp16 has 10 mantissa bits: on [1024,2048) the ulp is exactly 1.0, so the
# single f32->fp16 output rounding of (q + 1536) lands on the integer grid.
MAGIC = 1536.0
N_WARMUP = 32

_CACHE = {}


def _build_nc():
    import concourse.bass as bass
    import concourse.mybir as mybir
    import concourse.tile as tile
    from concourse.masks import make_identity

    F32 = mybir.dt.float32
    F16 = mybir.dt.float16
    ALU = mybir.AluOpType
    ACTF = mybir.ActivationFunctionType
    AX = mybir.AxisListType

    nc = bass.Bass("TRN2", target_bir_lowering=False, debug=False)

    x_d = nc.dram_tensor("x", [TC, IN_F], F16, kind="ExternalInput").ap()
    wb_d = nc.dram_tensor("wb", [OC, WROW], F16, kind="ExternalInput").ap()
    out_d = nc.dram_tensor("out", [OC, TC], F16, kind="ExternalOutput").ap()

    x3 = x_d.rearrange("(q p) i -> p q i", p=128)     # [128, NT, IN_F]
    w3 = wb_d.rearrange("(r p) i -> p r i", p=128)    # [128, NO, WROW]

    with tile.TileContext(nc) as tc:
        with (
            tc.tile_pool(name="raw", bufs=1) as raw,
            tc.tile_pool(name="qb", bufs=1) as qb,
            tc.tile_pool(name="small", bufs=1) as small,
            tc.tile_pool(name="ob", bufs=2) as obp,
            tc.tile_pool(name="mmps", bufs=4, space="PSUM") as mmps,
            tc.tile_pool(name="stps", bufs=2, space="PSUM") as stps,
            tc.tile_pool(name="auxps", bufs=1, space="PSUM") as auxps,
        ):
            x_sb = raw.tile([128, NT, IN_F], F16, tag="x_sb")
            w_sb = raw.tile([128, NO, WROW], F16, tag="w_sb")
            xq = qb.tile([128, NT, IN_F], F16, tag="xq")      # values +1536
            wq = qb.tile([128, NO, IN_F], F16, tag="wq")      # values +1536
            xqT = qb.tile([128, KB, TC], F16, tag="xqT")      # true ints
            wqT = qb.tile([128, KB, OC], F16, tag="wqT")      # true ints
            bcx = qb.tile([128, TC], F32, tag="bcx")          # sx/16129 bcast
            row_sb = qb.tile([1, TC], F32, tag="row_sb")
            ones1 = qb.tile([1, 128], F32, tag="ones1")
            ident_f16 = qb.tile([128, 128], F16, tag="ident_f16")

            # fp16 reduce outputs: max|fp16| is exactly representable in
            # fp16, and an all-2-byte instruction lets the DVE engage its
            # 2x/4x perf modes.
            xmax = small.tile([128, NT], F16, tag="xmax")
            xden = small.tile([128, NT], F32, tag="xden")
            xinv = small.tile([128, NT], F32, tag="xinv")
            wmax = small.tile([128, NO], F16, tag="wmax")
            wmaxf = small.tile([128, NO], F32, tag="wmaxf")   # f32 for TS scalars
            wmaxh = small.tile([128, 2], F16, tag="wmaxh")    # w3 half-maxes
            wden = small.tile([128, NO], F32, tag="wden")
            winv = small.tile([128, NO], F32, tag="winv")
            bias2 = small.tile([128, NO], F32, tag="bias2")

            ps = [mmps.tile([128, TC], F32, tag="ps", name=f"ps{m}")
                  for m in range(NO)]
            st = [stps.tile([128, 1024], F16, tag="st", name=f"st{i}")
                  for i in range(2)]
            rowps = auxps.tile([128, TC], F16, tag="rowps")   # xmax row stage
            bcps = auxps.tile([128, TC], F32, tag="bcps")     # bcx broadcast

            # ---- constants ----
            nc.gpsimd.memset(ones1, 1.0)
            make_identity(nc, ident_f16)
            # preload the ACT activation table during the load phase so the
            # first real ACTIVATE doesn't eat the 1.3us ACT_TABLE_LOAD
            nc.scalar.activation(out=row_sb[0:1, 0:128], in_=ones1,
                                 func=ACTF.Copy, scale=1.0, bias=0.0)

            # ---- loads: x tile pairs first, w tiles after, w3 split ----
            for q in range(0, NT, 2):
                nc.sync.dma_start(out=x_sb[:, q:q + 2], in_=x3[:, q:q + 2])
            for r in range(NO - 1):
                nc.sync.dma_start(out=w_sb[:, r:r + 1], in_=w3[:, r:r + 1])
            nc.sync.dma_start(out=w_sb[:, 3, 0:512], in_=w3[:, 3, 0:512])
            nc.sync.dma_start(out=w_sb[:, 3, 512:WROW], in_=w3[:, 3, 512:WROW])

            # ---- PE warmup: ramp the clock while loads stream ----
            for i in range(N_WARMUP):
                nc.tensor.transpose(st[0][:, 0:128], ident_f16, ident_f16)

            def x_red(q):
                nc.vector.tensor_reduce(
                    out=xmax[:, q:q + 1], in_=x_sb[:, q, :], axis=AX.X,
                    op=ALU.max, apply_absolute_value=True)
                nc.vector.tensor_scalar(
                    out=xden[:, q:q + 1], in0=xmax[:, q:q + 1],
                    scalar1=INV127, scalar2=EPS, op0=ALU.mult, op1=ALU.add)
                nc.vector.reciprocal(out=xinv[:, q:q + 1], in_=xden[:, q:q + 1])

            def x_quant_act(q, c0, c1):
                nc.scalar.activation(
                    out=xq[:, q, c0:c1], in_=x_sb[:, q, c0:c1], func=ACTF.Copy,
                    scale=xinv[:, q:q + 1], bias=MAGIC)

            def x_quant_pool(q, c0, c1):
                nc.gpsimd.tensor_scalar(
                    out=xq[:, q, c0:c1], in0=x_sb[:, q, c0:c1],
                    scalar1=xinv[:, q:q + 1], scalar2=MAGIC,
                    op0=ALU.mult, op1=ALU.add)

            def x_T(q):
                # all 8 contraction blocks of token-tile q into one st bank
                bank = st[q % 2]
                for k in range(KB):
                    nc.tensor.transpose(
                        bank[:, k * 128:(k + 1) * 128],
                        xq[:, q, k * 128:(k + 1) * 128], ident_f16)
                return bank

            def x_ev(q, bank, eng):
                # full-bank evict: st bank -> xqT[:, :, q-block], de-bias
                out_ap = xqT[:, 0:KB, q * 128:(q + 1) * 128]
                in_ap = bank.rearrange("p (k c) -> p k c", c=128)
                if eng == "act":
                    nc.scalar.activation(out=out_ap, in_=in_ap,
                                         func=ACTF.Copy, scale=1.0, bias=-MAGIC)
                else:
                    nc.vector.tensor_scalar(out=out_ap, in0=in_ap,
                                            scalar1=-MAGIC, scalar2=None,
                                            op0=ALU.add)

            def w_red(r):
                nc.vector.tensor_reduce(
                    out=wmax[:, r:r + 1], in_=w_sb[:, r, 0:IN_F], axis=AX.X,
                    op=ALU.max, apply_absolute_value=True)

            def w_den(r):
                nc.vector.tensor_scalar(
                    out=wden[:, r:r + 1], in0=wmax[:, r:r + 1],
                    scalar1=INV127, scalar2=EPS, op0=ALU.mult, op1=ALU.add)
                nc.vector.reciprocal(out=winv[:, r:r + 1], in_=wden[:, r:r + 1])
                nc.vector.tensor_copy(out=wmaxf[:, r:r + 1], in_=wmax[:, r:r + 1])

            def w_quant(r, c0, c1, eng):
                if eng == "act":
                    nc.scalar.activation(
                        out=wq[:, r, c0:c1], in_=w_sb[:, r, c0:c1],
                        func=ACTF.Copy, scale=winv[:, r:r + 1], bias=MAGIC)
                else:
                    nc.gpsimd.tensor_scalar(
                        out=wq[:, r, c0:c1], in0=w_sb[:, r, c0:c1],
                        scalar1=winv[:, r:r + 1], scalar2=MAGIC,
                        op0=ALU.mult, op1=ALU.add)

            def w_T(r):
                bank = st[r % 2]
                for k in range(KB):
                    nc.tensor.transpose(
                        bank[:, k * 128:(k + 1) * 128],
                        wq[:, r, k * 128:(k + 1) * 128], ident_f16)
                return bank

            def w_ev(r, bank, eng):
                out_ap = wqT[:, 0:KB, r * 128:(r + 1) * 128]
                in_ap = bank.rearrange("p (k c) -> p k c", c=128)
                if eng == "act":
                    nc.scalar.activation(out=out_ap, in_=in_ap,
                                         func=ACTF.Copy, scale=1.0, bias=-MAGIC)
                else:
                    nc.vector.tensor_scalar(out=out_ap, in0=in_ap,
                                            scalar1=-MAGIC, scalar2=None,
                                            op0=ALU.add)

            def mm_group(m):
                for k in range(KB):
                    nc.tensor.matmul(
                        ps[m], lhsT=wqT[:, k, m * 128:(m + 1) * 128],
                        rhs=xqT[:, k, :], start=(k == 0), stop=(k == KB - 1))

            def out_evict(m):
                otmp = obp.tile([128, TC], F16, tag="otmp", name=f"otmp{m}")
                nc.vector.tensor_tensor(out=otmp, in0=ps[m], in1=bcx,
                                        op=ALU.mult)
                osb = obp.tile([128, TC], F16, tag="osb", name=f"osb{m}")
                nc.gpsimd.tensor_scalar(
                    out=osb, in0=otmp, scalar1=wmaxf[:, m:m + 1],
                    scalar2=bias2[:, m:m + 1], op0=ALU.mult, op1=ALU.add)
                nc.sync.dma_start(out=out_d[m * 128:(m + 1) * 128, :], in_=osb)

            # ================= pipelined emission (approx time order) ======
            # --- x chains ---
            x_red(0)
            x_quant_act(0, 0, 512)
            x_quant_pool(0, 512, IN_F)
            x_T(0)
            x_red(1)
            x_quant_act(1, 0, 512)
            x_quant_pool(1, 512, IN_F)
            x_ev(0, st[0], "act")
            x_T(1)
            x_red(2)
            x_quant_act(2, 0, 512)
            x_quant_pool(2, 512, IN_F)
            x_ev(1, st[1], "dve")
            x_T(2)
            x_red(3)
            x_quant_act(3, 0, 512)
            x_quant_pool(3, 512, IN_F)
            x_ev(2, st[0], "act")
            x_T(3)
            # token-scale row: 4 single-column PE transposes into rowps
            for q in range(NT):
                nc.tensor.transpose(
                    rowps[0:1, q * 128:(q + 1) * 128], xmax[:, q:q + 1],
                    ident_f16)
            nc.scalar.activation(
                out=row_sb, in_=rowps[0:1, :], func=ACTF.Copy,
                scale=INV16129, bias=0.0)
            nc.tensor.matmul(bcps, lhsT=ones1, rhs=row_sb,
                             start=True, stop=True)
            x_ev(3, st[1], "dve")
            nc.vector.tensor_copy(out=bcx, in_=bcps)

            # --- w chains ---
            w_red(0)
            w_den(0)
            w_quant(0, 0, 512, "act")
            w_quant(0, 512, IN_F, "pool")
            nc.gpsimd.tensor_copy(out=bias2[:, 0:1], in_=w_sb[:, 0, IN_F:WROW])
            w_T(0)
            w_red(1)
            w_den(1)
            w_ev(0, st[0], "act")
            w_quant(1, 0, 512, "act")
            w_quant(1, 512, IN_F, "pool")
            nc.gpsimd.tensor_copy(out=bias2[:, 1:2], in_=w_sb[:, 1, IN_F:WROW])
            w_T(1)
            mm_group(0)
            w_red(2)
            w_den(2)
            w_ev(1, st[1], "dve")
            w_quant(2, 0, 512, "act")
            w_quant(2, 512, IN_F, "pool")
            nc.gpsimd.tensor_copy(out=bias2[:, 2:3], in_=w_sb[:, 2, IN_F:WROW])
            w_T(2)
            mm_group(1)
            out_evict(0)
            # w3: reduce halves on DVE, den on DVE, quant ACT || Pool
            nc.vector.tensor_reduce(
                out=wmaxh[:, 0:1], in_=w_sb[:, 3, 0:512], axis=AX.X,
                op=ALU.max, apply_absolute_value=True)
            nc.vector.tensor_reduce(
                out=wmaxh[:, 1:2], in_=w_sb[:, 3, 512:IN_F], axis=AX.X,
                op=ALU.max, apply_absolute_value=True)
            nc.vector.tensor_tensor(
                out=wmax[:, 3:4], in0=wmaxh[:, 0:1], in1=wmaxh[:, 1:2],
                op=ALU.max)
            w_den(3)
            w_ev(2, st[0], "act")
            w_quant(3, 0, 512, "act")
            w_quant(3, 512, IN_F, "pool")
            nc.gpsimd.tensor_copy(out=bias2[:, 3:4], in_=w_sb[:, 3, IN_F:WROW])
            w_T(3)
            mm_group(2)
            out_evict(1)
            w_ev(3, st[1], "dve")
            mm_group(3)
            out_evict(2)
            out_evict(3)

    _split_multiwaits(nc)
    return nc


def _split_multiwaits(nc):
    """Hoist all but one wait of any multi-wait instruction into standalone
    EventSemaphore instructions (the ISA carries one wait per instruction)."""
    import concourse.mybir as mybir

    fn = nc.m.functions[0]
    ctr = [0]
    for blk in fn.blocks:
        insts = list(blk.instructions)
        changed = False
        out = []
        for inst in insts:
            si = inst.sync_info
            waits = list(si.on_wait or []) if si is not None else []
            if len(waits) > 1:
                for w in waits[:-1]:
                    ctr[0] += 1
                    es = mybir.InstEventSemaphore(
                        name=f"I-eswait-{ctr[0]}", engine=inst.engine,
                        ins=[], outs=[],
                    )
                    es.sync_info = mybir.SyncInfo(on_wait=[w], on_update=[])
                    out.append(es)
                    nc.register_instruction(es)
                inst.sync_info = mybir.SyncInfo(
                    on_wait=[waits[-1]], on_update=list(si.on_update or []),
                )
                changed = True
            out.append(inst)
        if changed:
            blk.instructions = out


def get_nc():
    if "nc" not in _CACHE:
        _CACHE["nc"] = _build_nc()
    return _CACHE["nc"]


def make_in_maps(x, weight, bias):
    xf = np.asarray(x, dtype=np.float32).reshape(T, IN_F).astype(np.float16)
    w = np.asarray(weight, dtype=np.float32).astype(np.float16)
    b = np.asarray(bias, dtype=np.float32).astype(np.float16)
    wb = np.concatenate([w, b[:, None]], axis=1)
    in_maps = []
    for c in range(M_SHARDS * N_SHARDS):
        im, jn = divmod(c, N_SHARDS)
        in_maps.append({
            "x": np.ascontiguousarray(xf[im * TC:(im + 1) * TC]),
            "wb": np.ascontiguousarray(wb[jn * OC:(jn + 1) * OC]),
        })
    return in_maps


def assemble(results):
    y = np.empty((T, OUT_F), dtype=np.float32)
    for c in range(M_SHARDS * N_SHARDS):
        im, jn = divmod(c, N_SHARDS)
        y[im * TC:(im + 1) * TC, jn * OC:(jn + 1) * OC] = \
            results[c]["out"].T.astype(np.float32)
    return y.reshape(B, S, OUT_F)


def run(x, weight, bias, **spmd_kwargs):
    from concourse.bass_utils import run_bass_kernel_spmd

    nc = get_nc()
    in_maps = make_in_maps(x, weight, bias)
    res = run_bass_kernel_spmd(nc, in_maps, core_ids=list(range(8)), **spmd_kwargs)
    return assemble(res.results), res


def kernel(x, weight, bias):
    y, _ = run(x, weight, bias)
    return y
